# revision 1
# baseline (speedup 1.0000x reference)
"""Trainium2 Bass kernel for nn_Attention_90744069030375.

Row/column-permuted masked causal attention.  Steady state is PE-bound:
~147us (cost model) / ~152us (HW, extrapolated from the paired-A/B-
measured ~165us variant minus the removed mixed-tile lo pass) per core
vs the 542us baseline.

Reference (per batch, S=2048, D=1024):
    scores = (q @ k^T) * scale; bias = -1e9 * max(pad_i, pad_j, triu)
    attn = softmax(scores + bias); out = attn @ k     (v = k)

Key numerics: for a padded query row (mask[i]=False) every logit gets
-1e9; in fp32 ulp(1e9)=64 so `scores - 1e9` collapses the row onto a
64-wide grid and softmax becomes uniform over the top bucket.  Bucket
membership needs |score error| << distance-to-bucket-boundary, hence
high-precision QK^T for padded rows.  Valid rows are ordinary masked
softmax (masked entries underflow to weight exactly 0) and tolerate
1-pass fp16.

Design: per batch, permute rows AND columns valid-first (softmax is
row-wise, so row permutation commutes; keys/values permuted together so
the valid x valid block becomes standard causal attention in permuted
space).  Tiles of 128 query rows:
  - t < VT  (pure valid):  1-pass fp16 QK over cols <= 128(t+1) only
    (causal compaction -- exact: skipped cols have weight exactly 0),
    triangular -1e9 bias on the diagonal block, flash softmax, PV over
    (t+1) key blocks.
  - VT <= t < MIXHI (mixed valid/padded): 1-pass fp16 QK, exact
    per-element bias streamed as u8 (covers causal+pad for valid rows,
    all -1e9 for padded rows), exp path, full PV.  The ~1k padded rows
    in this region see hard bucket flips at 1-pass precision, adding
    ~5e-3 error (total measured 1.27e-2 vs the 2e-2 gate).
  - t >= MIXHI (pure padded): SINGLE fp16 QK pass; the reference's
    uniform-over-top-bucket softmax is reproduced on DVE with a clamp
    ramp of width h at the bucket's lower edge (fl is monotone, so the
    collapsed max is fl(rowmax - 1e9); no exp needed).

VT = floor(min_b V_b/128), MIXHI = ceil(max_b V_b/128) are data-driven
compile-time constants (program cached per (VT, MIXHI)).  Sharding:
data-parallel, one batch per NeuronCore, no collectives.
"""

import numpy as np
import ml_dtypes

import concourse.bass as bass
import concourse.bacc as bacc
import concourse.mybir as mybir
from concourse.bass_utils import run_bass_kernel_spmd
from concourse.tile import TileContext

B, S, D = 8, 2048, 1024
P = 128                 # partitions / tile rows
NQ = S // P             # 16 query row-tiles
ND = D // P             # 8 contraction chunks
NJ = S // 512           # 4 key column banks of 512
F16 = mybir.dt.float16
F32 = mybir.dt.float32
F8E5 = mybir.dt.float8e5


RAMP_H = 0.025


def build_bass(VT, MIXHI, reps=1, sc_bufs=6, pv_bufs=2, h=RAMP_H):
    """VT: # pure-valid causal tiles; MIXHI: first pure-padded tile.

    Pure-padded tiles run a SINGLE fp16 QK pass; the reference's
    uniform-over-top-64-bucket softmax is reproduced with a soft ramp of
    width h at the bucket boundary (computed on DVE, no exp needed),
    which tolerates the 1-pass score error.  Mixed tiles (containing
    valid rows) keep the 2-pass fp16 QK + exact-bias exp path.
    """
    VB = VT * P                      # valid-block columns
    MIXN = MIXHI - VT                # number of mixed tiles
    MG = MIXN * P                    # mixed-region rows (lo-pass q cols)

    nc = bacc.Bacc()
    qT = nc.dram_tensor("qT", [D, S], F16, kind="ExternalInput")
    kT = nc.dram_tensor("kT", [D, S], F16, kind="ExternalInput")
    kpv = nc.dram_tensor("kpv", [S, D], F16, kind="ExternalInput")
    tribias = nc.dram_tensor("tribias", [P, P], F32, kind="ExternalInput")
    biasmix = (nc.dram_tensor("biasmix", [MIXN * P, S], mybir.dt.uint8,
                              kind="ExternalInput") if MIXN else None)
    out = nc.dram_tensor("out", [S, D], F32, kind="ExternalOutput")

    # processing order: V0 first (needs almost no data), then alternate
    # padded (heavy) and valid (light) tiles; mixed tiles last.
    valid_ts = list(range(VT))
    padded_ts = list(range(MIXHI, NQ))
    mixed_ts = list(range(VT, MIXHI))
    order = []
    vi_, pi_ = 0, 0
    if valid_ts:
        order.append(("V", valid_ts[0])); vi_ = 1
    while vi_ < len(valid_ts) or pi_ < len(padded_ts):
        if pi_ < len(padded_ts):
            order.append(("P", padded_ts[pi_])); pi_ += 1
        if vi_ < len(valid_ts):
            order.append(("V", valid_ts[vi_])); vi_ += 1
    order += [("M", t) for t in mixed_ts]

    with TileContext(nc) as tc:
        with (
            tc.tile_pool(name="weights", bufs=1) as wpool,
            tc.tile_pool(name="work", bufs=2) as work,
            tc.tile_pool(name="stats", bufs=3) as stats,
            tc.tile_pool(name="scores", bufs=sc_bufs, space="PSUM") as scores_pool,
            tc.tile_pool(name="pv", bufs=pv_bufs, space="PSUM") as pv_pool,
        ):
            # ---- persistent operands (merged tiles, one slot per group) --
            qT_all = wpool.tile([P, ND * S], F16, tag="qT")
            kT_all = wpool.tile([P, ND * S], F16, tag="kT")
            kpv_all = wpool.tile([P, NQ * D], F16, tag="kpv")
            trib = wpool.tile([P, P], F32, tag="trib")

            def qslice(d, lo, hi):      # qT_all view, global row range
                return qT_all[:, d * S + lo: d * S + hi]

            def kslice(d, lo, hi):
                return kT_all[:, d * S + lo: d * S + hi]

            # DMA issue order ~ consumption order, split across parallel
            # channels: SP carries kT n0/n1 + qT + out; the gpsimd SWDGE
            # channel carries kT n2/n3, kpv and the mixed-tile bias.  ACT
            # keeps only the attn xbar transposes (its engine time is
            # shared with activation compute).
            nc.sync.dma_start(out=trib, in_=tribias[:, :])
            for n in (2, 3):
                nsl = slice(n * 512, (n + 1) * 512)
                for d in range(ND):
                    dsl = slice(d * P, (d + 1) * P)
                    nc.gpsimd.dma_start(out=kslice(d, n * 512, (n + 1) * 512),
                                        in_=kT[dsl, nsl])
            for j in range(NQ):
                nc.gpsimd.dma_start(out=kpv_all[:, j * D:(j + 1) * D],
                                    in_=kpv[j * P:(j + 1) * P, :])
            first_rows = [(t * P, (t + 1) * P) for _, t in order[:3]]
            done = set(first_rows)
            rest = []
            for _, t in order[3:]:
                lo, hi = t * P, (t + 1) * P
                if (lo, hi) in done:
                    continue
                done.add((lo, hi))
                rest.append((lo, hi))

            def load_q(eng, lo, hi):
                for d in range(ND):
                    dsl = slice(d * P, (d + 1) * P)
                    eng.dma_start(out=qslice(d, lo, hi), in_=qT[dsl, lo:hi])

            # SP: kT n-major interleaved with q rows in consumption
            # order.  (ACT stays free for exp + xbar transposes -- its
            # HWDGE dispatch competes with activation compute.)
            lo0, hi0 = first_rows[0]
            for d in range(ND):
                dsl = slice(d * P, (d + 1) * P)
                nc.sync.dma_start(out=qslice(d, lo0, hi0), in_=qT[dsl, lo0:hi0])
                nc.sync.dma_start(out=kslice(d, 0, 512), in_=kT[dsl, 0:512])
            for (lo, hi) in first_rows[1:]:
                load_q(nc.sync, lo, hi)
            for d in range(ND):
                dsl = slice(d * P, (d + 1) * P)
                nc.sync.dma_start(out=kslice(d, 512, 1024), in_=kT[dsl, 512:1024])
            for (lo, hi) in rest:
                load_q(nc.sync, lo, hi)


            # ---- per-tile emission ---------------------------------------
            pending_pv = []

            def emit_tile(kind, t, group=None):
                lo, hi = t * P, (t + 1) * P
                if kind == "V":
                    CB = (t + 1) * P            # causal column extent
                    NB = (CB + 511) // 512      # banks in use
                else:
                    CB, NB = S, NJ
                bw = [min(CB, (n + 1) * 512) - n * 512 for n in range(NB)]

                if kind == "M":
                    bias_u8 = work.tile([P, S], mybir.dt.uint8, tag="bias_u8")
                    nc.gpsimd.dma_start(
                        out=bias_u8, in_=biasmix[lo - VB:hi - VB, :])
                    bias = work.tile([P, S], F32, tag="bias")
                    nc.vector.tensor_scalar_mul(bias, bias_u8, float(-1e9))

                # QK^T: fp16, hi pass (+ lo pass for general tiles).
                # `group` = how many PSUM banks share one pass over the q
                # d-chunks; smaller groups start before all kT banks land.
                G = group or NB
                sc = [scores_pool.tile([P, 512], F32, name=f"sc{n}", tag="sc")
                      for n in range(NB)]
                for g0 in range(0, NB, G):
                    ns = range(g0, min(g0 + G, NB))
                    for d in range(ND):
                        qh_d = qslice(d, lo, hi)
                        for n in ns:
                            nc.tensor.matmul(
                                sc[n][:, :bw[n]], qh_d,
                                kslice(d, n * 512, n * 512 + bw[n]),
                                start=(d == 0),
                                stop=(d == ND - 1))

                # bias, then per-bank row-max on raw fp32 scores
                pmax = stats.tile([P, NB], F32, tag="pmax")
                for n in range(NB):
                    if kind == "V":
                        dlo = t * P - n * 512   # diag block offset in bank n
                        if 0 <= dlo < bw[n]:
                            nc.vector.tensor_add(
                                sc[n][:, dlo:dlo + P],
                                sc[n][:, dlo:dlo + P], trib)
                    elif kind == "M":
                        nc.vector.tensor_add(
                            sc[n], sc[n], bias[:, n * 512:(n + 1) * 512])
                    nc.vector.reduce_max(
                        pmax[:, n:n + 1], sc[n][:, :bw[n]],
                        axis=mybir.AxisListType.X)

                attn = work.tile([P, S], F16, tag="attn")
                psums = stats.tile([P, NJ], F32, tag="psums")
                recip = stats.tile([P, 1], F32, tag="recip")
                if kind != "P":
                    negmax = stats.tile([P, 1], F32, tag="negmax")
                    nc.vector.reduce_max(
                        negmax, pmax, axis=mybir.AxisListType.X, negate=True)
                    # exp(x - rowmax) on ACT, fused row-sums
                    for n in range(NB):
                        nc.scalar.activation(
                            out=attn[:, n * 512:n * 512 + bw[n]],
                            in_=sc[n][:, :bw[n]],
                            func=mybir.ActivationFunctionType.Exp,
                            bias=negmax, scale=1.0,
                            accum_out=psums[:, n:n + 1])
                else:
                    # Reference semantics for an all-masked row: fl32
                    # collapse of (s - 1e9) onto a 64-wide grid, softmax
                    # uniform over the top bucket.  fl is monotone, so the
                    # collapsed max is fl(rowmax(s) - 1e9); weights are a
                    # clamp ramp of width h at the bucket's lower edge
                    # (max-collapsed - 32), computed entirely on DVE.
                    maxs = stats.tile([P, 1], F32, tag="negmax")
                    nc.vector.reduce_max(
                        maxs, pmax, axis=mybir.AxisListType.X)
                    cm = stats.tile([P, 1], F32, tag="cms")
                    nc.vector.tensor_scalar_add(cm, maxs, float(-1e9))
                    e3 = stats.tile([P, 1], F32, tag="e3")
                    # e3 = -(cm + 1e9) + 32 + h/2   (cm + 1e9 is exact)
                    nc.vector.tensor_scalar(
                        e3, cm, float(1e9), -1.0,
                        mybir.AluOpType.add, mybir.AluOpType.mult)
                    nc.vector.tensor_scalar_add(e3, e3, float(32.0 + h / 2))
                    ut = work.tile([P, 512], F32, tag="ut")
                    for n in range(NB):
                        nc.vector.tensor_scalar(
                            ut, sc[n], e3, float(1.0 / h),
                            mybir.AluOpType.add, mybir.AluOpType.mult)
                        nc.vector.tensor_scalar(
                            attn[:, n * 512:(n + 1) * 512], ut, 0.0, 1.0,
                            mybir.AluOpType.max, mybir.AluOpType.min)
                        nc.vector.reduce_sum(
                            psums[:, n:n + 1],
                            attn[:, n * 512:(n + 1) * 512],
                            axis=mybir.AxisListType.X)
                nc.vector.reduce_sum(
                    recip, psums[:, :NB], axis=mybir.AxisListType.X)
                nc.vector.reciprocal(recip, recip)

                # transpose attn for PV (DMA xbar, SBUF->SBUF fp16)
                attnT = work.tile([P, NQ, P], F16, tag="attnT", bufs=3)
                for n in range(NB):
                    nc.scalar.dma_start(
                        out=attnT[:, 4 * n:4 * n + bw[n] // P, :],
                        in_=attn[:, n * 512:n * 512 + bw[n]],
                        transpose=True)

                NKB = (t + 1) if kind == "V" else NQ   # PV key blocks

                def make_pv(lo=lo, attnT=attnT, recip=recip, NKB=NKB):
                    def emit_pv():
                        pv = [pv_pool.tile([P, 512], F32, name=f"pv{nn}",
                                           tag="pv") for nn in range(2)]
                        for jb in range(NKB):
                            lhsT = attnT[:, jb, :]
                            for nn in range(2):
                                nc.tensor.matmul(
                                    pv[nn], lhsT,
                                    kpv_all[:, jb * D + nn * 512:
                                            jb * D + (nn + 1) * 512],
                                    start=(jb == 0), stop=(jb == NKB - 1))
                        osb = work.tile([P, D], F32, name="osb", tag="osb",
                                        bufs=2)
                        for nn in range(2):
                            nc.vector.tensor_scalar_mul(
                                osb[:, nn * 512:(nn + 1) * 512], pv[nn], recip)
                        nc.sync.dma_start(out=out[lo:lo + P, :], in_=osb)
                    return emit_pv

                if len(pending_pv) == 2:
                    pending_pv.pop(0)()
                pending_pv.append(make_pv())

            for r in range(reps):
                for oi, (kind, t) in enumerate(order):
                    # first two heavy tiles: pair-grouped QK so bank 0/1
                    # matmuls start before kT banks 2/3 finish streaming
                    g = 2 if (r == 0 and kind != "V" and oi < 4) else None
                    emit_tile(kind, t, group=g)
            for f in pending_pv:
                f()

    return nc


_NC_CACHE = {}


def _get_nc(VT, MIXHI, reps=1):
    key = (VT, MIXHI, reps)
    if key not in _NC_CACHE:
        nc = build_bass(VT, MIXHI, reps=reps)
        if not nc.is_finalized():
            nc.finalize()
        _NC_CACHE[key] = nc
    return _NC_CACHE[key]


def plan_split(mask):
    Vs = mask.sum(1)
    VT = int(min(Vs) // P)
    MIXHI = int(-(-int(max(Vs)) // P))
    MIXHI = max(MIXHI, VT + 1) if MIXHI < NQ else MIXHI
    return VT, MIXHI


def make_in_maps(q, k, mask, scale, VT, MIXHI):
    f16 = ml_dtypes.float16 if hasattr(ml_dtypes, "float16") else np.float16
    VB, MIXN = VT * P, MIXHI - VT
    s = float(np.asarray(scale))
    tri = (np.triu(np.ones((P, P), np.float32), k=1) * np.float32(-1e9)
           ).astype(np.float32)
    in_maps, perms = [], []
    for b in range(B):
        vi = np.where(mask[b])[0]
        pi = np.where(~mask[b])[0]
        perm = np.concatenate([vi, pi])
        V = len(vi)
        perms.append(perm)
        qp = (q[b] * s).astype(np.float32)[perm]
        kp = k[b][perm].astype(np.float32)
        qh = qp.astype(f16)
        # mixed-tile bias: valid row r allows cols c <= r; padded rows none
        if MIXN:
            rows = np.arange(VB, MIXHI * P)
            cols = np.arange(S)
            allowed = (cols[None, :] <= rows[:, None]) & (rows[:, None] < V)
            bm = (~allowed).astype(np.uint8)
        in_map = {
            "qT": np.ascontiguousarray(qh.T),
            "kT": np.ascontiguousarray(kp.astype(f16).T),
            "kpv": np.ascontiguousarray(kp.astype(f16)),
            "tribias": tri,
        }
        if MIXN:
            in_map["biasmix"] = bm
        in_maps.append(in_map)
    return in_maps, perms


def kernel(q, k, mask, scale, _want_trace=False, _reps=1, **trace_kwargs):
    q, k, mask = np.asarray(q), np.asarray(k), np.asarray(mask)
    VT, MIXHI = plan_split(mask)
    nc = _get_nc(VT, MIXHI, reps=_reps)
    in_maps, perms = make_in_maps(q, k, mask, np.asarray(scale), VT, MIXHI)
    res = run_bass_kernel_spmd(
        nc, in_maps, list(range(B)), trace=_want_trace, **trace_kwargs)
    outs = np.empty((B, S, D), np.float32)
    for b in range(B):
        outs[b][perms[b]] = res.results[b]["out"].astype(np.float32)
    if _want_trace:
        return outs, res
    return outs

